# revision 13
# baseline (speedup 1.0000x reference)
"""GQA attention (RoPE, causal) for one TRN2 chip (8 NeuronCores).

Sharding: core d handles batch b = d//4 and kv-group g = d%4
(8 q heads + 1 kv head per core).  Each core computes its partial
output-projection contribution out_partial[b] (shape [S, H]); the host
sums the 4 partials per batch and adds bo.  No collectives.

v3 changes over v2:
  - fully fused pipeline: the projections of chunk c+1 and the
    out-projection of chunk c-1 are interleaved into the attention
    groups of chunk c, so the ACT engine (exp) always has slack and PE
    never sees a phase boundary.
  - causal si-trim: the diagonal 512-chunk's QK matmuls skip the
    fully-masked leading si columns (psum regions compacted per pair so
    each pair still needs ONE exp instruction).
  - V transposes moved from PE+PSUM to the DMA xbar
    (dma_start_transpose straight out of the KV evac tile).
  - PSUM plan (8 banks): qk [128,1024]x2 = 4, pv [65,512]x2 = 2,
    oproj/bc [128,512]x1 = 1, proj [128,512]x1 = 1.

Layout (per core, all matmul contractions on partitions):
  xT  [H, S]      : x[b] transposed on host, bf16, streamed per 512-chunk
  qt  [128, 512]  : q^T head-pair-major (rope'd), rotating per chunk
  kt2 [128, S]    : k^T rope'd, duplicated in both partition halves
  vones[j][128,65]: v (sj-major) with appended ones col (denom trick)
  scores^T [sj,si]: lhsT=kt2 chunk, rhs=qt chunk  (no transposes needed)
  exp (no max-subtraction; |scores/8| <~ 6 so exp is safe in fp32/bf16)
  PV: lhsT=[V|1] [sj,65], rhs=expS^T -> psum [65, si] = [attn^T; denom]
  out[s,o]: lhsT=attnT chunk, rhs=woT chunk, accumulated over m tiles.
"""

import sys

if "/opt/trn_rl_repo" not in sys.path:
    sys.path.insert(0, "/opt/trn_rl_repo")

import numpy as np
import ml_dtypes

bf16 = ml_dtypes.bfloat16

B = 2
S = 2048
H = 2048
N_HEADS = 32
KV_HEADS = 4
HEAD_DIM = 64
ROPE_THETA = 10000.0
N_CORES = 8
ML = 512          # q-head features per core (8 heads * 64)
CHUNK = 512       # si chunk width
SJB = 128         # sj block width
HB = 128          # h (contraction) tile
NHB = H // HB     # 16 contraction tiles


def build_graph(nc, tile_mod, mybir, seq=S):
    """Emit the per-core graph. seq can be shrunk for simulator tests."""
    fp32 = mybir.dt.float32
    bfl = mybir.dt.bfloat16

    nC = seq // CHUNK       # si chunks
    nJ = seq // SJB         # sj blocks
    nHB = NHB               # contraction tiles
    nMT = ML // 128         # q-feature partition tiles (head pairs)

    xT = nc.dram_tensor("xT", [H, seq], bfl, kind="ExternalInput")
    wq = nc.dram_tensor("wq", [H, ML], bfl, kind="ExternalInput")
    wkv = nc.dram_tensor("wkv", [H, 128], bfl, kind="ExternalInput")
    wo = nc.dram_tensor("wo", [ML, H], bfl, kind="ExternalInput")
    bq = nc.dram_tensor("bq", [128, nMT], fp32, kind="ExternalInput")
    bkv = nc.dram_tensor("bkv", [128, 1], fp32, kind="ExternalInput")
    cos2 = nc.dram_tensor("cos2", [128, seq], bfl, kind="ExternalInput")
    sinS = nc.dram_tensor("sinS", [128, seq], bfl, kind="ExternalInput")
    tri = nc.dram_tensor("tri", [128, 128], bfl, kind="ExternalInput")
    ones64 = nc.dram_tensor("ones64", [1, 64], bfl, kind="ExternalInput")
    out = nc.dram_tensor("out", [seq, H], bfl, kind="ExternalOutput")

    Exp = mybir.ActivationFunctionType.Exp
    Ident = mybir.ActivationFunctionType.Identity
    tc = tile_mod.TileContext(nc)
    with tc:
        from contextlib import ExitStack
        with tc.tile_pool(name="persist", bufs=1) as P, \
             tc.tile_pool(name="outb", bufs=3) as OB, \
             ExitStack() as inner:
            XP = inner.enter_context(tc.tile_pool(name="xcp", bufs=2))
            QP = inner.enter_context(tc.tile_pool(name="qtp", bufs=2))
            T = inner.enter_context(tc.tile_pool(name="tmp", bufs=2))
            EP = inner.enter_context(tc.tile_pool(name="expp", bufs=22))
            SM = inner.enter_context(tc.tile_pool(name="small", bufs=6))
            PS1 = inner.enter_context(
                tc.tile_pool(name="ps1", bufs=1, space="PSUM"))
            QKP = inner.enter_context(
                tc.tile_pool(name="qkps", bufs=2, space="PSUM"))
            PVP = inner.enter_context(
                tc.tile_pool(name="pvps", bufs=2, space="PSUM"))
            OPS = inner.enter_context(
                tc.tile_pool(name="ops", bufs=1, space="PSUM"))

            wq_t = P.tile([128, nHB * ML], bfl, tag="wq", name="wq_t")
            wkv_t = P.tile([128, nHB * 128], bfl, tag="wkv", name="wkv_t")
            wo_t = P.tile([128, nMT * H], bfl, tag="wo", name="wo_t")
            cos_t = P.tile([128, seq], bfl, tag="cos", name="cos_t")
            sin_t = P.tile([128, seq], bfl, tag="sin", name="sin_t")
            bq_t = P.tile([128, nMT], fp32, tag="bq", name="bq_t")
            bkv_t = P.tile([128, 1], fp32, tag="bkv", name="bkv_t")
            tri_t = P.tile([128, 128], bfl, tag="tri", name="tri_t")
            ones64_t = P.tile([1, 64], bfl, tag="ones64", name="ones64_t")
            kt2 = P.tile([128, seq], bfl, tag="kt2", name="kt2")
            at = [P.tile([128, seq], bfl, tag=f"at{mt}", name=f"at{mt}")
                  for mt in range(nMT)]
            vones = [P.tile([128, 65], bfl, tag=f"vo{j}", name=f"vo{j}")
                     for j in range(nJ)]

            xT_r = xT.ap().rearrange("(b p) s -> p b s", p=128)
            wq_src = wq.ap().rearrange("(b p) m -> p b m", p=128)
            wkv_src = wkv.ap().rearrange("(b p) m -> p b m", p=128)
            wo_src = wo.ap().rearrange("(b p) o -> p b o", p=128)

            xcs = {}    # chunk -> xc tile
            qts = {}    # chunk -> [qt tile per mt]

            def stage_x(c, slices):
                """Allocate + DMA chunk c of xT (bf16 [128, 16*512])."""
                xc_t = XP.tile([128, nHB * CHUNK], bfl, tag="xc",
                               name=f"xc{c}")
                xcs[c] = xc_t
                xv = xc_t[:].rearrange("p (b w) -> p b w", b=nHB)
                for b0, b1 in slices:
                    nc.sync.dma_start(xv[:, b0:b1, :],
                                      xT_r[:, b0:b1, CHUNK * c:CHUNK * (c + 1)])

            def rope_math(t0, cs, dst, nrow):
                """dst[0:nrow, :] = t0*cos + halfswap(t0*sinS), where sinS
                carries the rotate-half sign pattern."""
                rs = T.tile([128, CHUNK], bfl, tag="rs", name="rs")
                nc.vector.tensor_mul(rs[0:nrow, :], t0[0:nrow, :],
                                     sin_t[0:nrow, cs])
                r2 = T.tile([128, CHUNK], bfl, tag="r2", name="r2")
                for b in range(nrow // 64):
                    nc.vector.tensor_copy(r2[64 * b:64 * b + 32, :],
                                          rs[64 * b + 32:64 * b + 64, :])
                    nc.vector.tensor_copy(r2[64 * b + 32:64 * b + 64, :],
                                          rs[64 * b:64 * b + 32, :])
                t1 = T.tile([128, CHUNK], bfl, tag="t1", name="t1")
                nc.vector.tensor_mul(t1[0:nrow, :], t0[0:nrow, :],
                                     cos_t[0:nrow, cs])
                nc.vector.tensor_add(dst, t1[0:nrow, :], r2[0:nrow, :])

            def build_proj_units(c, mts=None):
                """Unit closures for chunk c's projections (KV + Q tiles).
                stage_x(c) must already have been emitted.  mts selects
                which pieces to build: None = KV + all Q."""
                units = []
                cs = slice(CHUNK * c, CHUNK * (c + 1))
                xc_t = xcs[c]
                if c not in qts:
                    qts[c] = [QP.tile([128, CHUNK], bfl, tag=f"qt{mt}",
                                      name=f"qt{c}_{mt}")
                              for mt in range(nMT)]

                kvst = {}

                def kv_mm(k0):
                    if k0 == 0:
                        kvst["ps"] = PS1.tile([128, CHUNK], fp32, tag="ps",
                                              name="pskv")
                    ps = kvst["ps"]
                    for hb in range(k0, k0 + 4):
                        nc.tensor.matmul(
                            ps[:], wkv_t[:, 128 * hb:128 * hb + 128],
                            xc_t[:, CHUNK * hb:CHUNK * (hb + 1)],
                            start=(hb == 0), stop=(hb == nHB - 1))

                def kv_fin():
                    ps = kvst["ps"]
                    t0 = T.tile([128, CHUNK], bfl, tag="t0", name="t0")
                    kvst["t0"] = t0
                    nc.scalar.activation(t0[:], ps[:], Ident,
                                         bias=bkv_t[:, 0:1])
                    rope_math(t0, cs, kt2[0:64, cs], 64)
                    nc.vector.tensor_copy(kt2[64:128, cs], kt2[0:64, cs])

                def vtr(jl):
                    # DMA-xbar transpose of v^T [64,128] -> vones[j][:,0:64]
                    j = 4 * c + jl
                    nc.sync.dma_start_transpose(
                        vones[j][:, 0:64],
                        kvst["t0"][64:128, 128 * jl:128 * jl + 128])

                if mts is None or "kv" in mts:
                    for k0 in (0, 4, 8, 12):
                        units.append(lambda k0=k0: kv_mm(k0))
                    units.append(kv_fin)
                    for jl in range(4):
                        units.append(lambda jl=jl: vtr(jl))

                for mt in (range(nMT) if mts is None
                           else [m for m in mts if m != "kv"]):
                    qst = {}

                    def q_mm(k0, mt=mt, qst=qst):
                        if k0 == 0:
                            qst["ps"] = PS1.tile([128, CHUNK], fp32, tag="ps",
                                                 name="psq")
                        ps = qst["ps"]
                        for hb in range(k0, k0 + 4):
                            nc.tensor.matmul(
                                ps[:],
                                wq_t[:, ML * hb + 128 * mt:
                                     ML * hb + 128 * mt + 128],
                                xc_t[:, CHUNK * hb:CHUNK * (hb + 1)],
                                start=(hb == 0), stop=(hb == nHB - 1))

                    def q_fin(mt=mt, qst=qst):
                        ps = qst["ps"]
                        t0 = T.tile([128, CHUNK], bfl, tag="t0q", name="t0q")
                        nc.scalar.activation(t0[:], ps[:], Ident,
                                             bias=bq_t[:, mt:mt + 1])
                        rope_math(t0, cs, qts[c][mt][:, :], 128)

                    for k0 in (0, 4, 8, 12):
                        units.append(lambda k0=k0, f=q_mm: f(k0))
                    units.append(q_fin)
                return units

            def build_qk_units(c, mt):
                """QK+exp(+tri) unit closures for group (c, mt); returns
                (emap, units) where emap[hh][jb] = (ew, col, soff, w)."""
                njb = 4 * c + 4
                qt_c = qts[c][mt]
                emap = [{}, {}]
                units = []

                def unit_full(p, hh, pbase):
                    qsl = slice(pbase, pbase + 64)
                    qw = QKP.tile([128, 2 * CHUNK], fp32, tag="qk", name="qw")
                    for i, jb in enumerate((2 * p, 2 * p + 1)):
                        js = slice(128 * jb, 128 * jb + 128)
                        nc.tensor.matmul(
                            qw[:, CHUNK * i:CHUNK * (i + 1)],
                            kt2[qsl, js], qt_c[qsl, :],
                            start=True, stop=True, tile_position=(pbase, 0))
                    ew = EP.tile([128, 2 * CHUNK], bfl, tag="e", name="ew")
                    nc.scalar.activation(ew[:], qw[:], Exp, scale=0.125)
                    emap[hh][2 * p] = (ew, 0, 0, CHUNK)
                    emap[hh][2 * p + 1] = (ew, CHUNK, 0, CHUNK)

                def unit_d0(hh, pbase):
                    # diagonal pair (4c, 4c+1): widths 512, 384
                    qsl = slice(pbase, pbase + 64)
                    jb = 4 * c
                    qw = QKP.tile([128, 2 * CHUNK], fp32, tag="qk",
                                  name="qwd0")
                    nc.tensor.matmul(qw[:, 0:512],
                                     kt2[qsl, 128 * jb:128 * jb + 128],
                                     qt_c[qsl, :], start=True, stop=True,
                                     tile_position=(pbase, 0))
                    nc.tensor.matmul(qw[:, 512:896],
                                     kt2[qsl, 128 * jb + 128:128 * jb + 256],
                                     qt_c[qsl, 128:512], start=True,
                                     stop=True, tile_position=(pbase, 0))
                    ew = EP.tile([128, 2 * CHUNK], bfl, tag="e", name="ewd0")
                    nc.scalar.activation(ew[:, 0:896], qw[:, 0:896], Exp,
                                         scale=0.125)
                    nc.vector.tensor_mul(ew[:, 0:128], ew[:, 0:128], tri_t[:])
                    nc.vector.tensor_mul(ew[:, 512:640], ew[:, 512:640],
                                         tri_t[:])
                    emap[hh][jb] = (ew, 0, 0, 512)
                    emap[hh][jb + 1] = (ew, 512, 128, 384)

                def unit_d1(hh, pbase):
                    # diagonal pair (4c+2, 4c+3): widths 256, 128 (one bank)
                    qsl = slice(pbase, pbase + 64)
                    jb = 4 * c + 2
                    qw = QKP.tile([128, 2 * CHUNK], fp32, tag="qk",
                                  name="qwd1")
                    nc.tensor.matmul(qw[:, 0:256],
                                     kt2[qsl, 128 * jb:128 * jb + 128],
                                     qt_c[qsl, 256:512], start=True,
                                     stop=True, tile_position=(pbase, 0))
                    nc.tensor.matmul(qw[:, 256:384],
                                     kt2[qsl, 128 * jb + 128:128 * jb + 256],
                                     qt_c[qsl, 384:512], start=True,
                                     stop=True, tile_position=(pbase, 0))
                    ew = EP.tile([128, 2 * CHUNK], bfl, tag="e", name="ewd1")
                    nc.scalar.activation(ew[:, 0:384], qw[:, 0:384], Exp,
                                         scale=0.125)
                    nc.vector.tensor_mul(ew[:, 0:128], ew[:, 0:128], tri_t[:])
                    nc.vector.tensor_mul(ew[:, 256:384], ew[:, 256:384],
                                         tri_t[:])
                    emap[hh][jb] = (ew, 0, 256, 256)
                    emap[hh][jb + 1] = (ew, 256, 384, 128)

                for p in range(2 * c):
                    for hh, pbase in ((0, 0), (1, 64)):
                        units.append(
                            lambda p=p, hh=hh, pbase=pbase:
                            unit_full(p, hh, pbase))
                for hh, pbase in ((0, 0), (1, 64)):
                    units.append(lambda hh=hh, pbase=pbase: unit_d0(hh, pbase))
                for hh, pbase in ((0, 0), (1, 64)):
                    units.append(lambda hh=hh, pbase=pbase: unit_d1(hh, pbase))
                return emap, units

            def build_pv_units(c, mt, emap):
                """PV accumulation + divide closures for group (c, mt)."""
                cs = slice(CHUNK * c, CHUNK * (c + 1))
                njb = 4 * c + 4
                pvs = [PVP.tile([65, CHUNK], fp32, tag="pv", name="pv0"),
                       PVP.tile([65, CHUNK], fp32, tag="pv", name="pv1")]
                rbs = [None, None]
                units = []

                def pv_mm(hh, p):
                    for jb in (2 * p, 2 * p + 1):
                        ew, col, soff, w = emap[hh][jb]
                        nc.tensor.matmul(
                            pvs[hh][:, soff:soff + w],
                            vones[jb][:, 0:65], ew[:, col:col + w],
                            start=(jb == 0), stop=(jb == njb - 1))

                def recip(hh):
                    rb = SM.tile([1, CHUNK], bfl, tag="rb", name="rb")
                    with nc.allow_low_precision(
                            reason="bf16 softmax denom recip; ~0.4% "
                                   "noise well inside the 2e-2 gate"):
                        nc.vector.reciprocal(rb[:], pvs[hh][64:65, :])
                    rbs[hh] = rb

                def divide(hh):
                    # broadcast the reciprocal row across 64 partitions on
                    # the (otherwise idle) GPSIMD engine, then one DVE mul.
                    rbb = SM.tile([64, CHUNK], bfl, tag="bcs", name="rbb")
                    nc.gpsimd.partition_broadcast(rbb[:], rbs[hh][:],
                                                  channels=64)
                    nc.vector.tensor_mul(at[mt][64 * hh:64 * hh + 64, cs],
                                         pvs[hh][0:64, :], rbb[:])

                # divide(hh) directly after recip(hh): releases pvs[hh] a
                # half-group earlier, so the next group's PV psum rotation
                # never waits on an end-of-group DVE chain.
                for p in range(njb // 2):
                    units.append(lambda p=p: pv_mm(0, p))
                units.append(lambda: recip(0))
                units.append(lambda: divide(0))
                for p in range(njb // 2):
                    units.append(lambda p=p: pv_mm(1, p))
                units.append(lambda: recip(1))
                units.append(lambda: divide(1))
                return units

            def build_oproj_units(c, pool=None, tail=False):
                """Out-projection closures for chunk c (16 blocks)."""
                units = []
                obs = {}
                pool_ = pool if pool is not None else OPS

                def block(st, oc):
                    sit = 4 * c + st
                    ss = slice(128 * sit, 128 * sit + 128)
                    if oc == 0:
                        obs[st] = OB.tile([128, H], bfl, tag="ob", name="ob")
                    po = pool_.tile([128, CHUNK], fp32, tag="po", name="po")
                    for mt in range(nMT):
                        nc.tensor.matmul(
                            po[:], at[mt][:, ss],
                            wo_t[:, H * mt + CHUNK * oc:
                                 H * mt + CHUNK * (oc + 1)],
                            start=(mt == 0), stop=(mt == nMT - 1))
                    dst = obs[st][:, CHUNK * oc:CHUNK * (oc + 1)]
                    if tail and (st * 4 + oc) % 2 == 1:
                        # tail: ACT is idle — split evacuations across both
                        nc.scalar.activation(dst, po[:], Ident)
                    else:
                        nc.vector.tensor_copy(dst, po[:])
                    if oc == 3:
                        nc.sync.dma_start(out.ap()[ss, :], obs[st][:])

                for st in range(4):
                    for oc in range(4):
                        units.append(lambda st=st, oc=oc: block(st, oc))
                return units

            def merge(u1, u2):
                """Proportional order-preserving round-robin, u1 first."""
                n1, n2 = len(u1), len(u2)
                res, i, j = [], 0, 0
                while i < n1 or j < n2:
                    if j >= n2 or (i < n1 and i * n2 <= j * n1):
                        res.append(u1[i]); i += 1
                    else:
                        res.append(u2[j]); j += 1
                return res

            # ---------------- head: DMAs + proj(0) ------------------------
            wkvv = wkv_t[:].rearrange("p (b m) -> p b m", b=nHB)
            nc.sync.dma_start(wkvv[:, 0:4, :], wkv_src[:, 0:4, :])
            xc0 = XP.tile([128, nHB * CHUNK], bfl, tag="xc", name="xc0")
            xcs[0] = xc0
            xc0v = xc0[:].rearrange("p (b w) -> p b w", b=nHB)
            nc.sync.dma_start(xc0v[:, 0:4, :], xT_r[:, 0:4, 0:CHUNK])
            nc.sync.dma_start(wkvv[:, 4:16, :], wkv_src[:, 4:16, :])
            nc.sync.dma_start(xc0v[:, 4:10, :], xT_r[:, 4:10, 0:CHUNK])
            nc.sync.dma_start(xc0v[:, 10:16, :], xT_r[:, 10:16, 0:CHUNK])
            wqv = wq_t[:].rearrange("p (b m) -> p b m", b=nHB)
            nc.sync.dma_start(wqv[:, 0:8, :], wq_src[:, 0:8, :])
            nc.sync.dma_start(wqv[:, 8:16, :], wq_src[:, 8:16, :])
            nc.sync.dma_start(cos_t[:], cos2.ap())
            nc.sync.dma_start(sin_t[:], sinS.ap())
            nc.sync.dma_start(bq_t[:], bq.ap())
            nc.sync.dma_start(bkv_t[:], bkv.ap())
            nc.sync.dma_start(tri_t[:], tri.ap())
            nc.sync.dma_start(ones64_t[:], ones64.ap())
            for j in range(nJ):
                nc.vector.memset(vones[j][:, 64:65], 1.0)
            if nC > 1:
                stage_x(1, [(0, 8), (8, 16)])
            nc.sync.dma_start(
                wo_t[:].rearrange("p (b o) -> p b o", b=nMT), wo_src)

            for u in build_proj_units(0):
                u()

            # ---------------- fused attention pipeline --------------------
            groups = [(c, mt) for c in range(nC) for mt in range(nMT)]
            proj_next = {}   # chunk -> pending proj unit list
            pend, qk0 = build_qk_units(*groups[0])
            for u in qk0:
                u()
            for i, (c, mt) in enumerate(groups):
                if i + 1 < len(groups):
                    nxt, qk_u = build_qk_units(*groups[i + 1])
                else:
                    nxt, qk_u = None, []
                extras = []
                if mt == 0 and c + 1 < nC:
                    if c + 2 < nC:
                        stage_x(c + 2, [(0, 8), (8, 16)])
                    proj_next[c + 1] = build_proj_units(c + 1)
                if c + 1 < nC:
                    pu = proj_next[c + 1]
                    q0 = (len(pu) * mt) // nMT
                    q1 = (len(pu) * (mt + 1)) // nMT
                    extras += pu[q0:q1]
                if c > 0:
                    if mt == 0:
                        proj_next[f"op{c - 1}"] = build_oproj_units(c - 1)
                    ou = proj_next[f"op{c - 1}"]
                    extras += ou[4 * mt:4 * (mt + 1)]
                body = merge(build_pv_units(c, mt, pend), extras)
                for u in merge(qk_u, body):
                    u()
                pend = nxt
            inner.close()
            # tail: attention pools released — triple-buffered psum and
            # both evac engines keep the last out-projection stall-free.
            with tc.tile_pool(name="ops2", bufs=3, space="PSUM") as OPS2:
                for u in build_oproj_units(nC - 1, pool=OPS2, tail=True):
                    u()
    return nc


# ---------------------------------------------------------------------------
# host side
# ---------------------------------------------------------------------------

def _rope_tables(seq):
    inv_freq = 1.0 / (ROPE_THETA ** (np.arange(0, HEAD_DIM, 2, dtype=np.float32)
                                     / HEAD_DIM))
    t = np.arange(seq, dtype=np.float32)
    freqs = np.outer(t, inv_freq)                       # [S, 32]
    emb = np.concatenate([freqs, freqs], axis=-1)       # [S, 64]
    cos_t = np.cos(emb).astype(np.float32)
    sin_t = np.sin(emb).astype(np.float32)
    cos2 = np.tile(cos_t.T, (2, 1)).copy()              # [128, S]
    # "shuffled" sign layout: row r holds the multiplier that, after the
    # half-swap copy (rows r <-> r^32 within each 64-block), lands the
    # correct signed sin on the output row: +sin for r%64<32, -sin above.
    sgn = np.where(np.arange(HEAD_DIM) < HEAD_DIM // 2, 1.0, -1.0).astype(np.float32)
    sinS = np.tile((sin_t * sgn).T, (2, 1)).copy()      # [128, S]
    return cos2, sinS


def host_inputs(x, Wq, bq, Wk, bk, Wv, bv, Wo, seq=S):
    """Build in_maps for the 8 cores."""
    cos2, sinS = _rope_tables(seq)
    cos2 = cos2.astype(bf16)
    sinS = sinS.astype(bf16)
    r = np.arange(128)[:, None]
    cc = np.arange(128)[None, :]
    tri = (cc >= r).astype(np.float32).astype(bf16)     # [128, 128]
    ones64 = np.ones((1, 64), dtype=np.float32).astype(bf16)
    xTb = [np.ascontiguousarray(x[b, :seq, :].T).astype(bf16) for b in range(B)]
    in_maps = []
    for d in range(N_CORES):
        b, g = d // 4, d % 4
        wq_s = np.ascontiguousarray(Wq[ML * g:ML * (g + 1), :].T).astype(bf16)
        wk_s = np.ascontiguousarray(Wk[64 * g:64 * (g + 1), :].T).astype(bf16)
        wv_s = np.ascontiguousarray(Wv[64 * g:64 * (g + 1), :].T).astype(bf16)
        wkv_s = np.concatenate([wk_s, wv_s], axis=1)
        wo_s = np.ascontiguousarray(Wo[:, ML * g:ML * (g + 1)].T).astype(bf16)
        bq_s = np.ascontiguousarray(
            bq[ML * g:ML * (g + 1)].reshape(4, 128).T).astype(np.float32)
        bkv_s = np.concatenate([bk[64 * g:64 * (g + 1)],
                                bv[64 * g:64 * (g + 1)]]).reshape(128, 1)
        in_maps.append({
            "xT": xTb[b], "wq": wq_s, "wkv": wkv_s, "wo": wo_s,
            "bq": bq_s, "bkv": np.ascontiguousarray(bkv_s, dtype=np.float32),
            "cos2": cos2[:, :seq], "sinS": sinS[:, :seq], "tri": tri,
            "ones64": ones64,
        })
    return in_maps


_NC = None


def _get_nc():
    global _NC
    if _NC is None:
        import concourse.tile as tile_mod
        from concourse import bacc, mybir
        nc = bacc.Bacc("TRN2", target_bir_lowering=False, debug=False,
                       num_devices=N_CORES)
        build_graph(nc, tile_mod, mybir)
        nc.compile()
        _NC = nc
    return _NC


def kernel(**inputs):
    from concourse import bass_utils
    nc = _get_nc()
    x = np.asarray(inputs["x"], dtype=np.float32)
    in_maps = host_inputs(
        x, np.asarray(inputs["Wq"], np.float32), np.asarray(inputs["bq"], np.float32),
        np.asarray(inputs["Wk"], np.float32), np.asarray(inputs["bk"], np.float32),
        np.asarray(inputs["Wv"], np.float32), np.asarray(inputs["bv"], np.float32),
        np.asarray(inputs["Wo"], np.float32))
    res = bass_utils.run_bass_kernel_spmd(nc, in_maps, core_ids=list(range(N_CORES)))
    bo = np.asarray(inputs["bo"], np.float32)
    out = np.empty((B, S, H), dtype=np.float32)
    for b in range(B):
        acc = res.results[4 * b]["out"].astype(np.float32).copy()
        for g in range(1, 4):
            acc += res.results[4 * b + g]["out"]
        out[b] = acc + bo[None, :]
    return out


# revision 14
# speedup vs baseline: 1.6052x; 1.6052x over previous
"""GQA attention (RoPE, causal) for one TRN2 chip (8 NeuronCores).

Sharding: core d handles batch b = d//4 and kv-group g = d%4
(8 q heads + 1 kv head per core).  Each core computes its partial
output-projection contribution out_partial[b] (shape [S, H]); the host
sums the 4 partials per batch and adds bo.  No collectives.

v3 changes over v2:
  - fully fused pipeline: the projections of chunk c+1 and the
    out-projection of chunk c-1 are interleaved into the attention
    groups of chunk c, so the ACT engine (exp) always has slack and PE
    never sees a phase boundary.
  - causal si-trim: the diagonal 512-chunk's QK matmuls skip the
    fully-masked leading si columns (psum regions compacted per pair so
    each pair still needs ONE exp instruction).
  - V transposes moved from PE+PSUM to the DMA xbar
    (dma_start_transpose straight out of the KV evac tile).
  - PSUM plan (8 banks): qk [128,1024]x2 = 4, pv [65,512]x2 = 2,
    oproj/bc [128,512]x1 = 1, proj [128,512]x1 = 1.

Layout (per core, all matmul contractions on partitions):
  xT  [H, S]      : x[b] transposed on host, bf16, streamed per 512-chunk
  qt  [128, 512]  : q^T head-pair-major (rope'd), rotating per chunk
  kt2 [128, S]    : k^T rope'd, duplicated in both partition halves
  vones[j][128,65]: v (sj-major) with appended ones col (denom trick)
  scores^T [sj,si]: lhsT=kt2 chunk, rhs=qt chunk  (no transposes needed)
  exp (no max-subtraction; |scores/8| <~ 6 so exp is safe in fp32/bf16)
  PV: lhsT=[V|1] [sj,65], rhs=expS^T -> psum [65, si] = [attn^T; denom]
  out[s,o]: lhsT=attnT chunk, rhs=woT chunk, accumulated over m tiles.
"""

import sys

if "/opt/trn_rl_repo" not in sys.path:
    sys.path.insert(0, "/opt/trn_rl_repo")

import numpy as np
import ml_dtypes

bf16 = ml_dtypes.bfloat16

B = 2
S = 2048
H = 2048
N_HEADS = 32
KV_HEADS = 4
HEAD_DIM = 64
ROPE_THETA = 10000.0
N_CORES = 8
ML = 512          # q-head features per core (8 heads * 64)
CHUNK = 512       # si chunk width
SJB = 128         # sj block width
HB = 128          # h (contraction) tile
NHB = H // HB     # 16 contraction tiles


def build_graph(nc, tile_mod, mybir, seq=S):
    """Emit the per-core graph. seq can be shrunk for simulator tests."""
    fp32 = mybir.dt.float32
    bfl = mybir.dt.bfloat16

    nC = seq // CHUNK       # si chunks
    nJ = seq // SJB         # sj blocks
    nHB = NHB               # contraction tiles
    nMT = ML // 128         # q-feature partition tiles (head pairs)

    xT = nc.dram_tensor("xT", [H, seq], bfl, kind="ExternalInput")
    wq = nc.dram_tensor("wq", [H, ML], bfl, kind="ExternalInput")
    wkv = nc.dram_tensor("wkv", [H, 128], bfl, kind="ExternalInput")
    wo = nc.dram_tensor("wo", [ML, H], bfl, kind="ExternalInput")
    bq = nc.dram_tensor("bq", [128, nMT], fp32, kind="ExternalInput")
    bkv = nc.dram_tensor("bkv", [128, 1], fp32, kind="ExternalInput")
    cos2 = nc.dram_tensor("cos2", [128, seq], bfl, kind="ExternalInput")
    sinS = nc.dram_tensor("sinS", [128, seq], bfl, kind="ExternalInput")
    tri = nc.dram_tensor("tri", [128, 128], bfl, kind="ExternalInput")
    ones64 = nc.dram_tensor("ones64", [1, 64], bfl, kind="ExternalInput")
    out = nc.dram_tensor("out", [seq, H], bfl, kind="ExternalOutput")

    Exp = mybir.ActivationFunctionType.Exp
    Ident = mybir.ActivationFunctionType.Identity
    tc = tile_mod.TileContext(nc)
    with tc:
        from contextlib import ExitStack
        with tc.tile_pool(name="persist", bufs=1) as P, \
             tc.tile_pool(name="outb", bufs=3) as OB, \
             ExitStack() as inner:
            XP = inner.enter_context(tc.tile_pool(name="xcp", bufs=2))
            QP = inner.enter_context(tc.tile_pool(name="qtp", bufs=2))
            T = inner.enter_context(tc.tile_pool(name="tmp", bufs=2))
            EP = inner.enter_context(tc.tile_pool(name="expp", bufs=22))
            SM = inner.enter_context(tc.tile_pool(name="small", bufs=6))
            PS1 = inner.enter_context(
                tc.tile_pool(name="ps1", bufs=1, space="PSUM"))
            QKP = inner.enter_context(
                tc.tile_pool(name="qkps", bufs=2, space="PSUM"))
            PVP = inner.enter_context(
                tc.tile_pool(name="pvps", bufs=2, space="PSUM"))
            OPS = inner.enter_context(
                tc.tile_pool(name="ops", bufs=1, space="PSUM"))

            wq_t = P.tile([128, nHB * ML], bfl, tag="wq", name="wq_t")
            wkv_t = P.tile([128, nHB * 128], bfl, tag="wkv", name="wkv_t")
            wo_t = P.tile([128, nMT * H], bfl, tag="wo", name="wo_t")
            cos_t = P.tile([128, seq], bfl, tag="cos", name="cos_t")
            sin_t = P.tile([128, seq], bfl, tag="sin", name="sin_t")
            bq_t = P.tile([128, nMT], fp32, tag="bq", name="bq_t")
            bkv_t = P.tile([128, 1], fp32, tag="bkv", name="bkv_t")
            tri_t = P.tile([128, 128], bfl, tag="tri", name="tri_t")
            ones64_t = P.tile([1, 64], bfl, tag="ones64", name="ones64_t")
            kt2 = P.tile([128, seq], bfl, tag="kt2", name="kt2")
            at = [P.tile([128, seq], bfl, tag=f"at{mt}", name=f"at{mt}")
                  for mt in range(nMT)]
            vones = [P.tile([128, 65], bfl, tag=f"vo{j}", name=f"vo{j}")
                     for j in range(nJ)]

            xT_r = xT.ap().rearrange("(b p) s -> p b s", p=128)
            wq_src = wq.ap().rearrange("(b p) m -> p b m", p=128)
            wkv_src = wkv.ap().rearrange("(b p) m -> p b m", p=128)
            wo_src = wo.ap().rearrange("(b p) o -> p b o", p=128)

            xcs = {}    # chunk -> xc tile
            qts = {}    # chunk -> [qt tile per mt]

            def stage_x(c, slices):
                """Allocate + DMA chunk c of xT (bf16 [128, 16*512])."""
                xc_t = XP.tile([128, nHB * CHUNK], bfl, tag="xc",
                               name=f"xc{c}")
                xcs[c] = xc_t
                xv = xc_t[:].rearrange("p (b w) -> p b w", b=nHB)
                for b0, b1 in slices:
                    nc.sync.dma_start(xv[:, b0:b1, :],
                                      xT_r[:, b0:b1, CHUNK * c:CHUNK * (c + 1)])

            def rope_math(t0, cs, dst, nrow):
                """dst[0:nrow, :] = t0*cos + halfswap(t0*sinS), where sinS
                carries the rotate-half sign pattern."""
                rs = T.tile([128, CHUNK], bfl, tag="rs", name="rs")
                nc.vector.tensor_mul(rs[0:nrow, :], t0[0:nrow, :],
                                     sin_t[0:nrow, cs])
                r2 = T.tile([128, CHUNK], bfl, tag="r2", name="r2")
                for b in range(nrow // 64):
                    nc.vector.tensor_copy(r2[64 * b:64 * b + 32, :],
                                          rs[64 * b + 32:64 * b + 64, :])
                    nc.vector.tensor_copy(r2[64 * b + 32:64 * b + 64, :],
                                          rs[64 * b:64 * b + 32, :])
                t1 = T.tile([128, CHUNK], bfl, tag="t1", name="t1")
                nc.vector.tensor_mul(t1[0:nrow, :], t0[0:nrow, :],
                                     cos_t[0:nrow, cs])
                nc.vector.tensor_add(dst, t1[0:nrow, :], r2[0:nrow, :])

            def build_proj_units(c, mts=None):
                """Unit closures for chunk c's projections (KV + Q tiles).
                stage_x(c) must already have been emitted.  mts selects
                which pieces to build: None = KV + all Q."""
                units = []
                cs = slice(CHUNK * c, CHUNK * (c + 1))
                xc_t = xcs[c]
                if c not in qts:
                    qts[c] = [QP.tile([128, CHUNK], bfl, tag=f"qt{mt}",
                                      name=f"qt{c}_{mt}")
                              for mt in range(nMT)]

                kvst = {}

                def kv_mm(k0):
                    if k0 == 0:
                        kvst["ps"] = PS1.tile([128, CHUNK], fp32, tag="ps",
                                              name="pskv")
                    ps = kvst["ps"]
                    for hb in range(k0, k0 + 4):
                        nc.tensor.matmul(
                            ps[:], wkv_t[:, 128 * hb:128 * hb + 128],
                            xc_t[:, CHUNK * hb:CHUNK * (hb + 1)],
                            start=(hb == 0), stop=(hb == nHB - 1))

                def kv_fin():
                    ps = kvst["ps"]
                    t0 = T.tile([128, CHUNK], bfl, tag="t0", name="t0")
                    kvst["t0"] = t0
                    nc.scalar.activation(t0[:], ps[:], Ident,
                                         bias=bkv_t[:, 0:1])
                    rope_math(t0, cs, kt2[0:64, cs], 64)
                    nc.vector.tensor_copy(kt2[64:128, cs], kt2[0:64, cs])

                def vtr(jl):
                    # DMA-xbar transpose of v^T [64,128] -> vones[j][:,0:64]
                    j = 4 * c + jl
                    nc.sync.dma_start_transpose(
                        vones[j][:, 0:64],
                        kvst["t0"][64:128, 128 * jl:128 * jl + 128])

                if mts is None or "kv" in mts:
                    for k0 in (0, 4, 8, 12):
                        units.append(lambda k0=k0: kv_mm(k0))
                    units.append(kv_fin)
                    for jl in range(4):
                        units.append(lambda jl=jl: vtr(jl))

                for mt in (range(nMT) if mts is None
                           else [m for m in mts if m != "kv"]):
                    qst = {}

                    def q_mm(k0, mt=mt, qst=qst):
                        if k0 == 0:
                            qst["ps"] = PS1.tile([128, CHUNK], fp32, tag="ps",
                                                 name="psq")
                        ps = qst["ps"]
                        for hb in range(k0, k0 + 4):
                            nc.tensor.matmul(
                                ps[:],
                                wq_t[:, ML * hb + 128 * mt:
                                     ML * hb + 128 * mt + 128],
                                xc_t[:, CHUNK * hb:CHUNK * (hb + 1)],
                                start=(hb == 0), stop=(hb == nHB - 1))

                    def q_fin(mt=mt, qst=qst):
                        ps = qst["ps"]
                        t0 = T.tile([128, CHUNK], bfl, tag="t0q", name="t0q")
                        nc.scalar.activation(t0[:], ps[:], Ident,
                                             bias=bq_t[:, mt:mt + 1])
                        rope_math(t0, cs, qts[c][mt][:, :], 128)

                    for k0 in (0, 4, 8, 12):
                        units.append(lambda k0=k0, f=q_mm: f(k0))
                    units.append(q_fin)
                return units

            def build_qk_units(c, mt):
                """QK+exp(+tri) unit closures for group (c, mt); returns
                (emap, units) where emap[hh][jb] = (ew, col, soff, w)."""
                njb = 4 * c + 4
                qt_c = qts[c][mt]
                emap = [{}, {}]
                units = []

                def unit_full(p, hh, pbase):
                    qsl = slice(pbase, pbase + 64)
                    qw = QKP.tile([128, 2 * CHUNK], fp32, tag="qk", name="qw")
                    for i, jb in enumerate((2 * p, 2 * p + 1)):
                        js = slice(128 * jb, 128 * jb + 128)
                        nc.tensor.matmul(
                            qw[:, CHUNK * i:CHUNK * (i + 1)],
                            kt2[qsl, js], qt_c[qsl, :],
                            start=True, stop=True, tile_position=(pbase, 0))
                    ew = EP.tile([128, 2 * CHUNK], bfl, tag="e", name="ew")
                    nc.scalar.activation(ew[:], qw[:], Exp, scale=0.125)
                    emap[hh][2 * p] = (ew, 0, 0, CHUNK)
                    emap[hh][2 * p + 1] = (ew, CHUNK, 0, CHUNK)

                def unit_d0(hh, pbase):
                    # diagonal pair (4c, 4c+1): widths 512, 384
                    qsl = slice(pbase, pbase + 64)
                    jb = 4 * c
                    qw = QKP.tile([128, 2 * CHUNK], fp32, tag="qk",
                                  name="qwd0")
                    nc.tensor.matmul(qw[:, 0:512],
                                     kt2[qsl, 128 * jb:128 * jb + 128],
                                     qt_c[qsl, :], start=True, stop=True,
                                     tile_position=(pbase, 0))
                    nc.tensor.matmul(qw[:, 512:896],
                                     kt2[qsl, 128 * jb + 128:128 * jb + 256],
                                     qt_c[qsl, 128:512], start=True,
                                     stop=True, tile_position=(pbase, 0))
                    ew = EP.tile([128, 2 * CHUNK], bfl, tag="e", name="ewd0")
                    nc.scalar.activation(ew[:, 0:896], qw[:, 0:896], Exp,
                                         scale=0.125)
                    nc.vector.tensor_mul(ew[:, 0:128], ew[:, 0:128], tri_t[:])
                    nc.vector.tensor_mul(ew[:, 512:640], ew[:, 512:640],
                                         tri_t[:])
                    emap[hh][jb] = (ew, 0, 0, 512)
                    emap[hh][jb + 1] = (ew, 512, 128, 384)

                def unit_d1(hh, pbase):
                    # diagonal pair (4c+2, 4c+3): widths 256, 128 (one bank)
                    qsl = slice(pbase, pbase + 64)
                    jb = 4 * c + 2
                    qw = QKP.tile([128, 2 * CHUNK], fp32, tag="qk",
                                  name="qwd1")
                    nc.tensor.matmul(qw[:, 0:256],
                                     kt2[qsl, 128 * jb:128 * jb + 128],
                                     qt_c[qsl, 256:512], start=True,
                                     stop=True, tile_position=(pbase, 0))
                    nc.tensor.matmul(qw[:, 256:384],
                                     kt2[qsl, 128 * jb + 128:128 * jb + 256],
                                     qt_c[qsl, 384:512], start=True,
                                     stop=True, tile_position=(pbase, 0))
                    ew = EP.tile([128, 2 * CHUNK], bfl, tag="e", name="ewd1")
                    nc.scalar.activation(ew[:, 0:384], qw[:, 0:384], Exp,
                                         scale=0.125)
                    nc.vector.tensor_mul(ew[:, 0:128], ew[:, 0:128], tri_t[:])
                    nc.vector.tensor_mul(ew[:, 256:384], ew[:, 256:384],
                                         tri_t[:])
                    emap[hh][jb] = (ew, 0, 256, 256)
                    emap[hh][jb + 1] = (ew, 256, 384, 128)

                for p in range(2 * c):
                    for hh, pbase in ((0, 0), (1, 64)):
                        units.append(
                            lambda p=p, hh=hh, pbase=pbase:
                            unit_full(p, hh, pbase))
                for hh, pbase in ((0, 0), (1, 64)):
                    units.append(lambda hh=hh, pbase=pbase: unit_d0(hh, pbase))
                for hh, pbase in ((0, 0), (1, 64)):
                    units.append(lambda hh=hh, pbase=pbase: unit_d1(hh, pbase))
                return emap, units

            def build_pv_units(c, mt, emap):
                """PV accumulation + divide closures for group (c, mt)."""
                cs = slice(CHUNK * c, CHUNK * (c + 1))
                njb = 4 * c + 4
                pvs = [PVP.tile([65, CHUNK], fp32, tag="pv", name="pv0"),
                       PVP.tile([65, CHUNK], fp32, tag="pv", name="pv1")]
                rbs = [None, None]
                units = []

                def pv_mm(hh, p):
                    for jb in (2 * p, 2 * p + 1):
                        ew, col, soff, w = emap[hh][jb]
                        nc.tensor.matmul(
                            pvs[hh][:, soff:soff + w],
                            vones[jb][:, 0:65], ew[:, col:col + w],
                            start=(jb == 0), stop=(jb == njb - 1))

                def recip(hh):
                    rb = SM.tile([1, CHUNK], bfl, tag="rb", name="rb")
                    with nc.allow_low_precision(
                            reason="bf16 softmax denom recip; ~0.4% "
                                   "noise well inside the 2e-2 gate"):
                        nc.vector.reciprocal(rb[:], pvs[hh][64:65, :])
                    rbs[hh] = rb

                def divide(hh):
                    # broadcast the reciprocal row across 64 partitions on
                    # the (otherwise idle) GPSIMD engine, then one DVE mul.
                    rbb = SM.tile([64, CHUNK], bfl, tag="bcs", name="rbb")
                    nc.gpsimd.partition_broadcast(rbb[:], rbs[hh][:],
                                                  channels=64)
                    nc.vector.tensor_mul(at[mt][64 * hh:64 * hh + 64, cs],
                                         pvs[hh][0:64, :], rbb[:])

                for p in range(njb // 2):
                    units.append(lambda p=p: pv_mm(0, p))
                units.append(lambda: recip(0))
                for p in range(njb // 2):
                    units.append(lambda p=p: pv_mm(1, p))
                units.append(lambda: recip(1))
                units.append(lambda: divide(0))
                units.append(lambda: divide(1))
                return units

            def build_oproj_units(c, pool=None, tail=False):
                """Out-projection closures for chunk c (16 blocks)."""
                units = []
                obs = {}
                pool_ = pool if pool is not None else OPS

                def block(st, oc):
                    sit = 4 * c + st
                    ss = slice(128 * sit, 128 * sit + 128)
                    if oc == 0:
                        obs[st] = OB.tile([128, H], bfl, tag="ob", name="ob")
                    po = pool_.tile([128, CHUNK], fp32, tag="po", name="po")
                    for mt in range(nMT):
                        nc.tensor.matmul(
                            po[:], at[mt][:, ss],
                            wo_t[:, H * mt + CHUNK * oc:
                                 H * mt + CHUNK * (oc + 1)],
                            start=(mt == 0), stop=(mt == nMT - 1))
                    dst = obs[st][:, CHUNK * oc:CHUNK * (oc + 1)]
                    if tail and (st * 4 + oc) % 2 == 1:
                        # tail: ACT is idle — split evacuations across both
                        nc.scalar.activation(dst, po[:], Ident)
                    else:
                        nc.vector.tensor_copy(dst, po[:])
                    if oc == 3:
                        nc.sync.dma_start(out.ap()[ss, :], obs[st][:])

                for st in range(4):
                    for oc in range(4):
                        units.append(lambda st=st, oc=oc: block(st, oc))
                return units

            def merge(u1, u2):
                """Proportional order-preserving round-robin, u1 first."""
                n1, n2 = len(u1), len(u2)
                res, i, j = [], 0, 0
                while i < n1 or j < n2:
                    if j >= n2 or (i < n1 and i * n2 <= j * n1):
                        res.append(u1[i]); i += 1
                    else:
                        res.append(u2[j]); j += 1
                return res

            # ---------------- head: DMAs + proj(0) ------------------------
            wkvv = wkv_t[:].rearrange("p (b m) -> p b m", b=nHB)
            nc.sync.dma_start(wkvv[:, 0:4, :], wkv_src[:, 0:4, :])
            xc0 = XP.tile([128, nHB * CHUNK], bfl, tag="xc", name="xc0")
            xcs[0] = xc0
            xc0v = xc0[:].rearrange("p (b w) -> p b w", b=nHB)
            nc.sync.dma_start(xc0v[:, 0:4, :], xT_r[:, 0:4, 0:CHUNK])
            nc.sync.dma_start(wkvv[:, 4:16, :], wkv_src[:, 4:16, :])
            nc.sync.dma_start(xc0v[:, 4:10, :], xT_r[:, 4:10, 0:CHUNK])
            nc.sync.dma_start(xc0v[:, 10:16, :], xT_r[:, 10:16, 0:CHUNK])
            wqv = wq_t[:].rearrange("p (b m) -> p b m", b=nHB)
            nc.sync.dma_start(wqv[:, 0:8, :], wq_src[:, 0:8, :])
            nc.sync.dma_start(wqv[:, 8:16, :], wq_src[:, 8:16, :])
            nc.sync.dma_start(cos_t[:], cos2.ap())
            nc.sync.dma_start(sin_t[:], sinS.ap())
            nc.sync.dma_start(bq_t[:], bq.ap())
            nc.sync.dma_start(bkv_t[:], bkv.ap())
            nc.sync.dma_start(tri_t[:], tri.ap())
            nc.sync.dma_start(ones64_t[:], ones64.ap())
            for j in range(nJ):
                nc.vector.memset(vones[j][:, 64:65], 1.0)
            if nC > 1:
                stage_x(1, [(0, 8), (8, 16)])
            nc.sync.dma_start(
                wo_t[:].rearrange("p (b o) -> p b o", b=nMT), wo_src)

            for u in build_proj_units(0):
                u()

            # ---------------- fused attention pipeline --------------------
            groups = [(c, mt) for c in range(nC) for mt in range(nMT)]
            proj_next = {}   # chunk -> pending proj unit list
            pend, qk0 = build_qk_units(*groups[0])
            for u in qk0:
                u()
            for i, (c, mt) in enumerate(groups):
                if i + 1 < len(groups):
                    nxt, qk_u = build_qk_units(*groups[i + 1])
                else:
                    nxt, qk_u = None, []
                extras = []
                if mt == 0 and c + 1 < nC:
                    if c + 2 < nC:
                        stage_x(c + 2, [(0, 8), (8, 16)])
                    proj_next[c + 1] = build_proj_units(c + 1)
                if c + 1 < nC:
                    pu = proj_next[c + 1]
                    q0 = (len(pu) * mt) // nMT
                    q1 = (len(pu) * (mt + 1)) // nMT
                    extras += pu[q0:q1]
                if c > 0:
                    if mt == 0:
                        proj_next[f"op{c - 1}"] = build_oproj_units(c - 1)
                    ou = proj_next[f"op{c - 1}"]
                    extras += ou[4 * mt:4 * (mt + 1)]
                body = merge(build_pv_units(c, mt, pend), extras)
                for u in merge(qk_u, body):
                    u()
                pend = nxt
            inner.close()
            # tail: attention pools released — triple-buffered psum and
            # both evac engines keep the last out-projection stall-free.
            with tc.tile_pool(name="ops2", bufs=3, space="PSUM") as OPS2:
                for u in build_oproj_units(nC - 1, pool=OPS2, tail=True):
                    u()
    return nc


# ---------------------------------------------------------------------------
# host side
# ---------------------------------------------------------------------------

def _rope_tables(seq):
    inv_freq = 1.0 / (ROPE_THETA ** (np.arange(0, HEAD_DIM, 2, dtype=np.float32)
                                     / HEAD_DIM))
    t = np.arange(seq, dtype=np.float32)
    freqs = np.outer(t, inv_freq)                       # [S, 32]
    emb = np.concatenate([freqs, freqs], axis=-1)       # [S, 64]
    cos_t = np.cos(emb).astype(np.float32)
    sin_t = np.sin(emb).astype(np.float32)
    cos2 = np.tile(cos_t.T, (2, 1)).copy()              # [128, S]
    # "shuffled" sign layout: row r holds the multiplier that, after the
    # half-swap copy (rows r <-> r^32 within each 64-block), lands the
    # correct signed sin on the output row: +sin for r%64<32, -sin above.
    sgn = np.where(np.arange(HEAD_DIM) < HEAD_DIM // 2, 1.0, -1.0).astype(np.float32)
    sinS = np.tile((sin_t * sgn).T, (2, 1)).copy()      # [128, S]
    return cos2, sinS


def host_inputs(x, Wq, bq, Wk, bk, Wv, bv, Wo, seq=S):
    """Build in_maps for the 8 cores."""
    cos2, sinS = _rope_tables(seq)
    cos2 = cos2.astype(bf16)
    sinS = sinS.astype(bf16)
    r = np.arange(128)[:, None]
    cc = np.arange(128)[None, :]
    tri = (cc >= r).astype(np.float32).astype(bf16)     # [128, 128]
    ones64 = np.ones((1, 64), dtype=np.float32).astype(bf16)
    xTb = [np.ascontiguousarray(x[b, :seq, :].T).astype(bf16) for b in range(B)]
    in_maps = []
    for d in range(N_CORES):
        b, g = d // 4, d % 4
        wq_s = np.ascontiguousarray(Wq[ML * g:ML * (g + 1), :].T).astype(bf16)
        wk_s = np.ascontiguousarray(Wk[64 * g:64 * (g + 1), :].T).astype(bf16)
        wv_s = np.ascontiguousarray(Wv[64 * g:64 * (g + 1), :].T).astype(bf16)
        wkv_s = np.concatenate([wk_s, wv_s], axis=1)
        wo_s = np.ascontiguousarray(Wo[:, ML * g:ML * (g + 1)].T).astype(bf16)
        bq_s = np.ascontiguousarray(
            bq[ML * g:ML * (g + 1)].reshape(4, 128).T).astype(np.float32)
        bkv_s = np.concatenate([bk[64 * g:64 * (g + 1)],
                                bv[64 * g:64 * (g + 1)]]).reshape(128, 1)
        in_maps.append({
            "xT": xTb[b], "wq": wq_s, "wkv": wkv_s, "wo": wo_s,
            "bq": bq_s, "bkv": np.ascontiguousarray(bkv_s, dtype=np.float32),
            "cos2": cos2[:, :seq], "sinS": sinS[:, :seq], "tri": tri,
            "ones64": ones64,
        })
    return in_maps


_NC = None


def _get_nc():
    global _NC
    if _NC is None:
        import concourse.tile as tile_mod
        from concourse import bacc, mybir
        nc = bacc.Bacc("TRN2", target_bir_lowering=False, debug=False,
                       num_devices=N_CORES)
        build_graph(nc, tile_mod, mybir)
        nc.compile()
        _NC = nc
    return _NC


def kernel(**inputs):
    from concourse import bass_utils
    nc = _get_nc()
    x = np.asarray(inputs["x"], dtype=np.float32)
    in_maps = host_inputs(
        x, np.asarray(inputs["Wq"], np.float32), np.asarray(inputs["bq"], np.float32),
        np.asarray(inputs["Wk"], np.float32), np.asarray(inputs["bk"], np.float32),
        np.asarray(inputs["Wv"], np.float32), np.asarray(inputs["bv"], np.float32),
        np.asarray(inputs["Wo"], np.float32))
    res = bass_utils.run_bass_kernel_spmd(nc, in_maps, core_ids=list(range(N_CORES)))
    bo = np.asarray(inputs["bo"], np.float32)
    out = np.empty((B, S, H), dtype=np.float32)
    for b in range(B):
        acc = res.results[4 * b]["out"].astype(np.float32).copy()
        for g in range(1, 4):
            acc += res.results[4 * b + g]["out"]
        out[b] = acc + bo[None, :]
    return out


# revision 15
# speedup vs baseline: 1.6805x; 1.0469x over previous
"""GQA attention (RoPE, causal) for one TRN2 chip (8 NeuronCores).

Sharding: core d handles batch b = d//4 and kv-group g = d%4
(8 q heads + 1 kv head per core).  Each core computes its partial
output-projection contribution out_partial[b] (shape [S, H]); the host
sums the 4 partials per batch and adds bo.  No collectives.

v3 changes over v2:
  - fully fused pipeline: the projections of chunk c+1 and the
    out-projection of chunk c-1 are interleaved into the attention
    groups of chunk c, so the ACT engine (exp) always has slack and PE
    never sees a phase boundary.
  - causal si-trim: the diagonal 512-chunk's QK matmuls skip the
    fully-masked leading si columns (psum regions compacted per pair so
    each pair still needs ONE exp instruction).
  - V transposes moved from PE+PSUM to the DMA xbar
    (dma_start_transpose straight out of the KV evac tile).
  - PSUM plan (8 banks): qk [128,1024]x2 = 4, pv [65,512]x2 = 2,
    oproj/bc [128,512]x1 = 1, proj [128,512]x1 = 1.

Layout (per core, all matmul contractions on partitions):
  xT  [H, S]      : x[b] transposed on host, bf16, streamed per 512-chunk
  qt  [128, 512]  : q^T head-pair-major (rope'd), rotating per chunk
  kt2 [128, S]    : k^T rope'd, duplicated in both partition halves
  vones[j][128,65]: v (sj-major) with appended ones col (denom trick)
  scores^T [sj,si]: lhsT=kt2 chunk, rhs=qt chunk  (no transposes needed)
  exp (no max-subtraction; |scores/8| <~ 6 so exp is safe in fp32/bf16)
  PV: lhsT=[V|1] [sj,65], rhs=expS^T -> psum [65, si] = [attn^T; denom]
  out[s,o]: lhsT=attnT chunk, rhs=woT chunk, accumulated over m tiles.
"""

import sys

if "/opt/trn_rl_repo" not in sys.path:
    sys.path.insert(0, "/opt/trn_rl_repo")

import numpy as np
import ml_dtypes

bf16 = ml_dtypes.bfloat16

B = 2
S = 2048
H = 2048
N_HEADS = 32
KV_HEADS = 4
HEAD_DIM = 64
ROPE_THETA = 10000.0
N_CORES = 8
ML = 512          # q-head features per core (8 heads * 64)
CHUNK = 512       # si chunk width
SJB = 128         # sj block width
HB = 128          # h (contraction) tile
NHB = H // HB     # 16 contraction tiles


def build_graph(nc, tile_mod, mybir, seq=S):
    """Emit the per-core graph. seq can be shrunk for simulator tests."""
    fp32 = mybir.dt.float32
    bfl = mybir.dt.bfloat16

    nC = seq // CHUNK       # si chunks
    nJ = seq // SJB         # sj blocks
    nHB = NHB               # contraction tiles
    nMT = ML // 128         # q-feature partition tiles (head pairs)

    xT = nc.dram_tensor("xT", [H, seq], bfl, kind="ExternalInput")
    wq = nc.dram_tensor("wq", [H, ML], bfl, kind="ExternalInput")
    wkv = nc.dram_tensor("wkv", [H, 128], bfl, kind="ExternalInput")
    wo = nc.dram_tensor("wo", [ML, H], bfl, kind="ExternalInput")
    bq = nc.dram_tensor("bq", [128, nMT], fp32, kind="ExternalInput")
    bkv = nc.dram_tensor("bkv", [128, 1], fp32, kind="ExternalInput")
    cos2 = nc.dram_tensor("cos2", [128, seq], bfl, kind="ExternalInput")
    sinS = nc.dram_tensor("sinS", [128, seq], bfl, kind="ExternalInput")
    tri = nc.dram_tensor("tri", [128, 128], bfl, kind="ExternalInput")
    ones64 = nc.dram_tensor("ones64", [1, 64], bfl, kind="ExternalInput")
    out = nc.dram_tensor("out", [seq, H], bfl, kind="ExternalOutput")

    Exp = mybir.ActivationFunctionType.Exp
    Ident = mybir.ActivationFunctionType.Identity
    tc = tile_mod.TileContext(nc)
    with tc:
        from contextlib import ExitStack
        with tc.tile_pool(name="persist", bufs=1) as P, \
             tc.tile_pool(name="outb", bufs=3) as OB, \
             ExitStack() as inner:
            XP = inner.enter_context(tc.tile_pool(name="xcp", bufs=2))
            QP = inner.enter_context(tc.tile_pool(name="qtp", bufs=2))
            T = inner.enter_context(tc.tile_pool(name="tmp", bufs=2))
            EP = inner.enter_context(tc.tile_pool(name="expp", bufs=22))
            SM = inner.enter_context(tc.tile_pool(name="small", bufs=6))
            PS1 = inner.enter_context(
                tc.tile_pool(name="ps1", bufs=1, space="PSUM"))
            QKP = inner.enter_context(
                tc.tile_pool(name="qkps", bufs=2, space="PSUM"))
            PVP = inner.enter_context(
                tc.tile_pool(name="pvps", bufs=2, space="PSUM"))
            OPS = inner.enter_context(
                tc.tile_pool(name="ops", bufs=1, space="PSUM"))

            wq_t = P.tile([128, nHB * ML], bfl, tag="wq", name="wq_t")
            wkv_t = P.tile([128, nHB * 128], bfl, tag="wkv", name="wkv_t")
            wo_t = P.tile([128, nMT * H], bfl, tag="wo", name="wo_t")
            cos_t = P.tile([128, seq], bfl, tag="cos", name="cos_t")
            sin_t = P.tile([128, seq], bfl, tag="sin", name="sin_t")
            bq_t = P.tile([128, nMT], fp32, tag="bq", name="bq_t")
            bkv_t = P.tile([128, 1], fp32, tag="bkv", name="bkv_t")
            tri_t = P.tile([128, 128], bfl, tag="tri", name="tri_t")
            ones64_t = P.tile([1, 64], bfl, tag="ones64", name="ones64_t")
            kt2 = P.tile([128, seq], bfl, tag="kt2", name="kt2")
            at = [P.tile([128, seq], bfl, tag=f"at{mt}", name=f"at{mt}")
                  for mt in range(nMT)]
            vones = [P.tile([128, 65], bfl, tag=f"vo{j}", name=f"vo{j}")
                     for j in range(nJ)]

            xT_r = xT.ap().rearrange("(b p) s -> p b s", p=128)
            wq_src = wq.ap().rearrange("(b p) m -> p b m", p=128)
            wkv_src = wkv.ap().rearrange("(b p) m -> p b m", p=128)
            wo_src = wo.ap().rearrange("(b p) o -> p b o", p=128)

            xcs = {}    # chunk -> xc tile
            qts = {}    # chunk -> [qt tile per mt]

            def stage_x(c, slices):
                """Allocate + DMA chunk c of xT (bf16 [128, 16*512])."""
                xc_t = XP.tile([128, nHB * CHUNK], bfl, tag="xc",
                               name=f"xc{c}")
                xcs[c] = xc_t
                xv = xc_t[:].rearrange("p (b w) -> p b w", b=nHB)
                for b0, b1 in slices:
                    nc.sync.dma_start(xv[:, b0:b1, :],
                                      xT_r[:, b0:b1, CHUNK * c:CHUNK * (c + 1)])

            def rope_math(t0, cs, dst, nrow):
                """dst[0:nrow, :] = t0*cos + halfswap(t0*sinS), where sinS
                carries the rotate-half sign pattern."""
                rs = T.tile([128, CHUNK], bfl, tag="rs", name="rs")
                nc.vector.tensor_mul(rs[0:nrow, :], t0[0:nrow, :],
                                     sin_t[0:nrow, cs])
                r2 = T.tile([128, CHUNK], bfl, tag="r2", name="r2")
                for b in range(nrow // 64):
                    nc.vector.tensor_copy(r2[64 * b:64 * b + 32, :],
                                          rs[64 * b + 32:64 * b + 64, :])
                    nc.vector.tensor_copy(r2[64 * b + 32:64 * b + 64, :],
                                          rs[64 * b:64 * b + 32, :])
                t1 = T.tile([128, CHUNK], bfl, tag="t1", name="t1")
                nc.vector.tensor_mul(t1[0:nrow, :], t0[0:nrow, :],
                                     cos_t[0:nrow, cs])
                nc.vector.tensor_add(dst, t1[0:nrow, :], r2[0:nrow, :])

            def build_proj_units(c, mts=None):
                """Unit closures for chunk c's projections (KV + Q tiles).
                stage_x(c) must already have been emitted.  mts selects
                which pieces to build: None = KV + all Q."""
                units = []
                cs = slice(CHUNK * c, CHUNK * (c + 1))
                xc_t = xcs[c]
                if c not in qts:
                    qts[c] = [QP.tile([128, CHUNK], bfl, tag=f"qt{mt}",
                                      name=f"qt{c}_{mt}")
                              for mt in range(nMT)]

                kvst = {}

                def kv_mm(k0):
                    if k0 == 0:
                        kvst["ps"] = PS1.tile([128, CHUNK], fp32, tag="ps",
                                              name="pskv")
                    ps = kvst["ps"]
                    for hb in range(k0, k0 + 4):
                        nc.tensor.matmul(
                            ps[:], wkv_t[:, 128 * hb:128 * hb + 128],
                            xc_t[:, CHUNK * hb:CHUNK * (hb + 1)],
                            start=(hb == 0), stop=(hb == nHB - 1))

                def kv_fin():
                    ps = kvst["ps"]
                    t0 = T.tile([128, CHUNK], bfl, tag="t0", name="t0")
                    kvst["t0"] = t0
                    nc.scalar.activation(t0[:], ps[:], Ident,
                                         bias=bkv_t[:, 0:1])
                    rope_math(t0, cs, kt2[0:64, cs], 64)
                    nc.vector.tensor_copy(kt2[64:128, cs], kt2[0:64, cs])

                def vtr(jl):
                    # DMA-xbar transpose of v^T [64,128] -> vones[j][:,0:64]
                    j = 4 * c + jl
                    nc.sync.dma_start_transpose(
                        vones[j][:, 0:64],
                        kvst["t0"][64:128, 128 * jl:128 * jl + 128])

                if mts is None or "kv" in mts:
                    for k0 in (0, 4, 8, 12):
                        units.append(lambda k0=k0: kv_mm(k0))
                    units.append(kv_fin)
                    for jl in range(4):
                        units.append(lambda jl=jl: vtr(jl))

                for mt in (range(nMT) if mts is None
                           else [m for m in mts if m != "kv"]):
                    qst = {}

                    def q_mm(k0, mt=mt, qst=qst):
                        if k0 == 0:
                            qst["ps"] = PS1.tile([128, CHUNK], fp32, tag="ps",
                                                 name="psq")
                        ps = qst["ps"]
                        for hb in range(k0, k0 + 4):
                            nc.tensor.matmul(
                                ps[:],
                                wq_t[:, ML * hb + 128 * mt:
                                     ML * hb + 128 * mt + 128],
                                xc_t[:, CHUNK * hb:CHUNK * (hb + 1)],
                                start=(hb == 0), stop=(hb == nHB - 1))

                    def q_fin(mt=mt, qst=qst):
                        ps = qst["ps"]
                        t0 = T.tile([128, CHUNK], bfl, tag="t0q", name="t0q")
                        nc.scalar.activation(t0[:], ps[:], Ident,
                                             bias=bq_t[:, mt:mt + 1])
                        rope_math(t0, cs, qts[c][mt][:, :], 128)

                    for k0 in (0, 4, 8, 12):
                        units.append(lambda k0=k0, f=q_mm: f(k0))
                    units.append(q_fin)
                return units

            def build_qk_units(c, mt):
                """QK+exp(+tri) unit closures for group (c, mt); returns
                (emap, units) where emap[hh][jb] = (ew, col, soff, w)."""
                njb = 4 * c + 4
                qt_c = qts[c][mt]
                emap = [{}, {}]
                units = []

                def unit_full(p, hh, pbase):
                    qsl = slice(pbase, pbase + 64)
                    qw = QKP.tile([128, 2 * CHUNK], fp32, tag="qk", name="qw")
                    for i, jb in enumerate((2 * p, 2 * p + 1)):
                        js = slice(128 * jb, 128 * jb + 128)
                        nc.tensor.matmul(
                            qw[:, CHUNK * i:CHUNK * (i + 1)],
                            kt2[qsl, js], qt_c[qsl, :],
                            start=True, stop=True, tile_position=(pbase, 0))
                    ew = EP.tile([128, 2 * CHUNK], bfl, tag="e", name="ew")
                    nc.scalar.activation(ew[:], qw[:], Exp, scale=0.125)
                    emap[hh][2 * p] = (ew, 0, 0, CHUNK)
                    emap[hh][2 * p + 1] = (ew, CHUNK, 0, CHUNK)

                def unit_d0(hh, pbase):
                    # diagonal pair (4c, 4c+1): widths 512, 384
                    qsl = slice(pbase, pbase + 64)
                    jb = 4 * c
                    qw = QKP.tile([128, 2 * CHUNK], fp32, tag="qk",
                                  name="qwd0")
                    nc.tensor.matmul(qw[:, 0:512],
                                     kt2[qsl, 128 * jb:128 * jb + 128],
                                     qt_c[qsl, :], start=True, stop=True,
                                     tile_position=(pbase, 0))
                    nc.tensor.matmul(qw[:, 512:896],
                                     kt2[qsl, 128 * jb + 128:128 * jb + 256],
                                     qt_c[qsl, 128:512], start=True,
                                     stop=True, tile_position=(pbase, 0))
                    ew = EP.tile([128, 2 * CHUNK], bfl, tag="e", name="ewd0")
                    nc.scalar.activation(ew[:, 0:896], qw[:, 0:896], Exp,
                                         scale=0.125)
                    nc.vector.tensor_mul(ew[:, 0:128], ew[:, 0:128], tri_t[:])
                    nc.vector.tensor_mul(ew[:, 512:640], ew[:, 512:640],
                                         tri_t[:])
                    emap[hh][jb] = (ew, 0, 0, 512)
                    emap[hh][jb + 1] = (ew, 512, 128, 384)

                def unit_d1(hh, pbase):
                    # diagonal pair (4c+2, 4c+3): widths 256, 128 (one bank)
                    qsl = slice(pbase, pbase + 64)
                    jb = 4 * c + 2
                    qw = QKP.tile([128, 2 * CHUNK], fp32, tag="qk",
                                  name="qwd1")
                    nc.tensor.matmul(qw[:, 0:256],
                                     kt2[qsl, 128 * jb:128 * jb + 128],
                                     qt_c[qsl, 256:512], start=True,
                                     stop=True, tile_position=(pbase, 0))
                    nc.tensor.matmul(qw[:, 256:384],
                                     kt2[qsl, 128 * jb + 128:128 * jb + 256],
                                     qt_c[qsl, 384:512], start=True,
                                     stop=True, tile_position=(pbase, 0))
                    ew = EP.tile([128, 2 * CHUNK], bfl, tag="e", name="ewd1")
                    nc.scalar.activation(ew[:, 0:384], qw[:, 0:384], Exp,
                                         scale=0.125)
                    nc.vector.tensor_mul(ew[:, 0:128], ew[:, 0:128], tri_t[:])
                    nc.vector.tensor_mul(ew[:, 256:384], ew[:, 256:384],
                                         tri_t[:])
                    emap[hh][jb] = (ew, 0, 256, 256)
                    emap[hh][jb + 1] = (ew, 256, 384, 128)

                for p in range(2 * c):
                    for hh, pbase in ((0, 0), (1, 64)):
                        units.append(
                            lambda p=p, hh=hh, pbase=pbase:
                            unit_full(p, hh, pbase))
                for hh, pbase in ((0, 0), (1, 64)):
                    units.append(lambda hh=hh, pbase=pbase: unit_d0(hh, pbase))
                for hh, pbase in ((0, 0), (1, 64)):
                    units.append(lambda hh=hh, pbase=pbase: unit_d1(hh, pbase))
                return emap, units

            def build_pv_units(c, mt, emap):
                """Flipped PV: out[si, d] = ew_block.T @ [V|1], streaming
                only 65 columns per (si-block, sj-block) with all 128
                output partitions live.  Divide becomes a per-partition
                scalar mul; at[mt] is written via an xbar transpose of the
                [si, d-pair] tile."""
                njb = 4 * c + 4
                # per head: [128 si, 4 si-blocks x 65]; col 65*sb+64 = denom
                pvs = [PVP.tile([128, 4 * 65], fp32, tag="pv", name="pv0"),
                       PVP.tile([128, 4 * 65], fp32, tag="pv", name="pv1")]
                pairs = [SM.tile([128, 128], bfl, tag="pt", name=f"pt{sb}")
                         for sb in range(4)]
                rcs = [None, None]
                units = []

                def pv_sb(hh, sb):
                    sit = 4 * c + sb
                    for jb in range(sit + 1):
                        ew, col, soff, w = emap[hh][jb]
                        lo = col + 128 * sb - soff
                        nc.tensor.matmul(
                            pvs[hh][:, 65 * sb:65 * sb + 65],
                            ew[:, lo:lo + 128], vones[jb][:, 0:65],
                            start=(jb == 0), stop=(jb == sit))

                def recip(hh):
                    rc = SM.tile([128, 4], fp32, tag="rb", name="rc")
                    dn = pvs[hh][:].rearrange("p (s x) -> p s x", x=65)
                    nc.vector.reciprocal(rc[:], dn[:, :, 64])
                    rcs[hh] = rc

                def divide(hh):
                    rc = rcs[hh]
                    for sb in range(4):
                        nc.vector.tensor_scalar_mul(
                            pairs[sb][:, 64 * hh:64 * hh + 64],
                            pvs[hh][:, 65 * sb:65 * sb + 64],
                            rc[:, sb:sb + 1])

                def trans():
                    for sb in range(4):
                        sit = 4 * c + sb
                        nc.sync.dma_start_transpose(
                            at[mt][:, 128 * sit:128 * sit + 128],
                            pairs[sb][:])

                for sb in range(4):
                    units.append(lambda sb=sb: pv_sb(0, sb))
                units.append(lambda: recip(0))
                units.append(lambda: divide(0))
                for sb in range(4):
                    units.append(lambda sb=sb: pv_sb(1, sb))
                units.append(lambda: recip(1))
                units.append(lambda: divide(1))
                units.append(trans)
                return units

            def build_oproj_units(c, pool=None, tail=False):
                """Out-projection closures for chunk c (16 blocks)."""
                units = []
                obs = {}
                pool_ = pool if pool is not None else OPS

                def block(st, oc):
                    sit = 4 * c + st
                    ss = slice(128 * sit, 128 * sit + 128)
                    if oc == 0:
                        obs[st] = OB.tile([128, H], bfl, tag="ob", name="ob")
                    po = pool_.tile([128, CHUNK], fp32, tag="po", name="po")
                    for mt in range(nMT):
                        nc.tensor.matmul(
                            po[:], at[mt][:, ss],
                            wo_t[:, H * mt + CHUNK * oc:
                                 H * mt + CHUNK * (oc + 1)],
                            start=(mt == 0), stop=(mt == nMT - 1))
                    dst = obs[st][:, CHUNK * oc:CHUNK * (oc + 1)]
                    if tail and (st * 4 + oc) % 2 == 1:
                        # tail: ACT is idle — split evacuations across both
                        nc.scalar.activation(dst, po[:], Ident)
                    else:
                        nc.vector.tensor_copy(dst, po[:])
                    if oc == 3:
                        nc.sync.dma_start(out.ap()[ss, :], obs[st][:])

                for st in range(4):
                    for oc in range(4):
                        units.append(lambda st=st, oc=oc: block(st, oc))
                return units

            def merge(u1, u2):
                """Proportional order-preserving round-robin, u1 first."""
                n1, n2 = len(u1), len(u2)
                res, i, j = [], 0, 0
                while i < n1 or j < n2:
                    if j >= n2 or (i < n1 and i * n2 <= j * n1):
                        res.append(u1[i]); i += 1
                    else:
                        res.append(u2[j]); j += 1
                return res

            # ---------------- head: DMAs + proj(0) ------------------------
            wkvv = wkv_t[:].rearrange("p (b m) -> p b m", b=nHB)
            nc.sync.dma_start(wkvv[:, 0:4, :], wkv_src[:, 0:4, :])
            xc0 = XP.tile([128, nHB * CHUNK], bfl, tag="xc", name="xc0")
            xcs[0] = xc0
            xc0v = xc0[:].rearrange("p (b w) -> p b w", b=nHB)
            nc.sync.dma_start(xc0v[:, 0:4, :], xT_r[:, 0:4, 0:CHUNK])
            nc.sync.dma_start(wkvv[:, 4:16, :], wkv_src[:, 4:16, :])
            nc.sync.dma_start(xc0v[:, 4:10, :], xT_r[:, 4:10, 0:CHUNK])
            nc.sync.dma_start(xc0v[:, 10:16, :], xT_r[:, 10:16, 0:CHUNK])
            wqv = wq_t[:].rearrange("p (b m) -> p b m", b=nHB)
            nc.sync.dma_start(wqv[:, 0:8, :], wq_src[:, 0:8, :])
            nc.sync.dma_start(wqv[:, 8:16, :], wq_src[:, 8:16, :])
            nc.sync.dma_start(cos_t[:], cos2.ap())
            nc.sync.dma_start(sin_t[:], sinS.ap())
            nc.sync.dma_start(bq_t[:], bq.ap())
            nc.sync.dma_start(bkv_t[:], bkv.ap())
            nc.sync.dma_start(tri_t[:], tri.ap())
            nc.sync.dma_start(ones64_t[:], ones64.ap())
            for j in range(nJ):
                nc.vector.memset(vones[j][:, 64:65], 1.0)
            if nC > 1:
                stage_x(1, [(0, 8), (8, 16)])
            nc.sync.dma_start(
                wo_t[:].rearrange("p (b o) -> p b o", b=nMT), wo_src)

            for u in build_proj_units(0):
                u()

            # ---------------- fused attention pipeline --------------------
            groups = [(c, mt) for c in range(nC) for mt in range(nMT)]
            proj_next = {}   # chunk -> pending proj unit list
            pend, qk0 = build_qk_units(*groups[0])
            for u in qk0:
                u()
            for i, (c, mt) in enumerate(groups):
                if i + 1 < len(groups):
                    nxt, qk_u = build_qk_units(*groups[i + 1])
                else:
                    nxt, qk_u = None, []
                extras = []
                if mt == 0 and c + 1 < nC:
                    if c + 2 < nC:
                        stage_x(c + 2, [(0, 8), (8, 16)])
                    proj_next[c + 1] = build_proj_units(c + 1)
                if c + 1 < nC:
                    pu = proj_next[c + 1]
                    q0 = (len(pu) * mt) // nMT
                    q1 = (len(pu) * (mt + 1)) // nMT
                    extras += pu[q0:q1]
                if c > 0:
                    if mt == 0:
                        proj_next[f"op{c - 1}"] = build_oproj_units(c - 1)
                    ou = proj_next[f"op{c - 1}"]
                    extras += ou[4 * mt:4 * (mt + 1)]
                body = merge(build_pv_units(c, mt, pend), extras)
                for u in merge(qk_u, body):
                    u()
                pend = nxt
            inner.close()
            # tail: attention pools released — triple-buffered psum and
            # both evac engines keep the last out-projection stall-free.
            with tc.tile_pool(name="ops2", bufs=3, space="PSUM") as OPS2:
                for u in build_oproj_units(nC - 1, pool=OPS2, tail=True):
                    u()
    return nc


# ---------------------------------------------------------------------------
# host side
# ---------------------------------------------------------------------------

def _rope_tables(seq):
    inv_freq = 1.0 / (ROPE_THETA ** (np.arange(0, HEAD_DIM, 2, dtype=np.float32)
                                     / HEAD_DIM))
    t = np.arange(seq, dtype=np.float32)
    freqs = np.outer(t, inv_freq)                       # [S, 32]
    emb = np.concatenate([freqs, freqs], axis=-1)       # [S, 64]
    cos_t = np.cos(emb).astype(np.float32)
    sin_t = np.sin(emb).astype(np.float32)
    cos2 = np.tile(cos_t.T, (2, 1)).copy()              # [128, S]
    # "shuffled" sign layout: row r holds the multiplier that, after the
    # half-swap copy (rows r <-> r^32 within each 64-block), lands the
    # correct signed sin on the output row: +sin for r%64<32, -sin above.
    sgn = np.where(np.arange(HEAD_DIM) < HEAD_DIM // 2, 1.0, -1.0).astype(np.float32)
    sinS = np.tile((sin_t * sgn).T, (2, 1)).copy()      # [128, S]
    return cos2, sinS


def host_inputs(x, Wq, bq, Wk, bk, Wv, bv, Wo, seq=S):
    """Build in_maps for the 8 cores."""
    cos2, sinS = _rope_tables(seq)
    cos2 = cos2.astype(bf16)
    sinS = sinS.astype(bf16)
    r = np.arange(128)[:, None]
    cc = np.arange(128)[None, :]
    tri = (cc >= r).astype(np.float32).astype(bf16)     # [128, 128]
    ones64 = np.ones((1, 64), dtype=np.float32).astype(bf16)
    xTb = [np.ascontiguousarray(x[b, :seq, :].T).astype(bf16) for b in range(B)]
    in_maps = []
    for d in range(N_CORES):
        b, g = d // 4, d % 4
        wq_s = np.ascontiguousarray(Wq[ML * g:ML * (g + 1), :].T).astype(bf16)
        wk_s = np.ascontiguousarray(Wk[64 * g:64 * (g + 1), :].T).astype(bf16)
        wv_s = np.ascontiguousarray(Wv[64 * g:64 * (g + 1), :].T).astype(bf16)
        wkv_s = np.concatenate([wk_s, wv_s], axis=1)
        wo_s = np.ascontiguousarray(Wo[:, ML * g:ML * (g + 1)].T).astype(bf16)
        bq_s = np.ascontiguousarray(
            bq[ML * g:ML * (g + 1)].reshape(4, 128).T).astype(np.float32)
        bkv_s = np.concatenate([bk[64 * g:64 * (g + 1)],
                                bv[64 * g:64 * (g + 1)]]).reshape(128, 1)
        in_maps.append({
            "xT": xTb[b], "wq": wq_s, "wkv": wkv_s, "wo": wo_s,
            "bq": bq_s, "bkv": np.ascontiguousarray(bkv_s, dtype=np.float32),
            "cos2": cos2[:, :seq], "sinS": sinS[:, :seq], "tri": tri,
            "ones64": ones64,
        })
    return in_maps


_NC = None


def _get_nc():
    global _NC
    if _NC is None:
        import concourse.tile as tile_mod
        from concourse import bacc, mybir
        nc = bacc.Bacc("TRN2", target_bir_lowering=False, debug=False,
                       num_devices=N_CORES)
        build_graph(nc, tile_mod, mybir)
        nc.compile()
        _NC = nc
    return _NC


def kernel(**inputs):
    from concourse import bass_utils
    nc = _get_nc()
    x = np.asarray(inputs["x"], dtype=np.float32)
    in_maps = host_inputs(
        x, np.asarray(inputs["Wq"], np.float32), np.asarray(inputs["bq"], np.float32),
        np.asarray(inputs["Wk"], np.float32), np.asarray(inputs["bk"], np.float32),
        np.asarray(inputs["Wv"], np.float32), np.asarray(inputs["bv"], np.float32),
        np.asarray(inputs["Wo"], np.float32))
    res = bass_utils.run_bass_kernel_spmd(nc, in_maps, core_ids=list(range(N_CORES)))
    bo = np.asarray(inputs["bo"], np.float32)
    out = np.empty((B, S, H), dtype=np.float32)
    for b in range(B):
        acc = res.results[4 * b]["out"].astype(np.float32).copy()
        for g in range(1, 4):
            acc += res.results[4 * b + g]["out"]
        out[b] = acc + bo[None, :]
    return out
